# revision 19
# baseline (speedup 1.0000x reference)
"""Trainium2 Bass kernel for nn_ContrastiveModel (CPC-style contrastive loss).

Math (per step s in (1, 2) with weights (W_s, b_s)):
    pred_s[m, l, c] = sum_p patient[m, l, p] * W_s[c, p] + b_s[c]      (l < L-s)
    l_neg_s[n, m, l] = sum_c timesteps[n, l+s, c] * pred_s[m, l, c]
    logits = concat_s(l_neg_s / T)   [N, N, (L-1)+(L-2)]
    y_trues = arange(N) broadcast    [N, sum]
    accs[s] = mean over (n,l) of all_m((m==n) | (l_neg[n,n,l] > l_neg[n,m,l]))

Sharding: everything is independent across the time axis l (pred is a
per-timestep linear map; the all-pairs einsum contracts channels only), so
l is sharded across the 8 cores: core k owns l in [128k, 128k+128), and
needs patient[:, l] plus timesteps[:, l+1..l+129] (query-side halo). No
replicated compute, no collectives. Outputs come back l-sharded and are
re-assembled on host; accs are scale-invariant under the 1/T scaling so
they are computed on host from the returned logits.

Dtypes: all matmuls run in float32r (TRN2 performance-fp32; ~1e-4 rel
err, 1 cycle/row when the moving dim is >= 256). PSUM accumulates fp32.

Key structure per core (128 l's in 32 blocks of 4):
  - pred matmuls: stationary W chunk [p,c], moving patient [p, (4l x m)]
    = 512-wide.
  - l_neg matmuls: one query timestep t[ii] = timesteps[:, lk+ii+1] pairs
    with step-1 pred at l=ii AND step-2 pred at l=ii-1. The pred->SBUF
    copies scatter into combined tiles comb[g] laid out [c, (j, s, m)]
    (g = ii//4, j = ii%4) so each l_neg matmul is stationary t[ii] x
    moving 256 (both steps at once) -- full float32r rate.

DMA: host pre-transposes shards so every load is one large contiguous-row
DMA (1 MiB patient block / 1 MiB 4-timestep query block / 1 MiB weights).
Loads ride the SP and ACT HWDGE rings; stores go via SWDGE so the three
rings run in parallel.
"""
import sys

sys.path.insert(0, "/opt/trn_rl_repo")
from contextlib import ExitStack

import numpy as np

import concourse.bass as bass
import concourse.tile as tile
from concourse import bacc, mybir
from concourse import bass_utils

N = 128          # patients (= queries = candidates)
L = 1024         # timesteps
C = 512          # channels (Ct == Cp)
TEMP = 0.07
NCORES = 8
LCH = L // NCORES    # l per core = 128
LB = LCH // 4        # l-blocks of 4 per core = 32

F32 = mybir.dt.float32
F32R = mybir.dt.float32r
BF16 = mybir.dt.bfloat16

LNEG_DT = "f32r"     # dtype of t / comb operands: "f32r" or "bf16"

_prog_cache = {}


def _build_program(lneg_dt_name=LNEG_DT, n_lb=LB, loop_n=None):
    """Build the SPMD Bass program (identical on every core). Returns nc.

    loop_n: if set, wrap the whole body in an on-device For_i repeat loop
    (for wall-clock timing through the ~100ms axon dispatch noise)."""
    lneg_dt = F32R if lneg_dt_name == "f32r" else BF16
    nc = bacc.Bacc("TRN2", target_bir_lowering=False, debug=False,
                   num_devices=NCORES)

    # DRAM inputs (per-core shapes; see prep_inputs for layouts)
    pt_d = nc.dram_tensor("pt", [LB, 128, 4, 4, 128], F32R,
                          kind="ExternalInput").ap()   # [lb, p_in, pc, l4, m]
    ts_d = nc.dram_tensor("ts", [LCH + 4, 128, 4, 128], lneg_dt,
                          kind="ExternalInput").ap()   # [ii, c_in, cc, n]
    w1_d = nc.dram_tensor("w1", [128, 4, 4, 128], F32R,
                          kind="ExternalInput").ap()   # [p_in, pc, cc, c_in]
    w2_d = nc.dram_tensor("w2", [128, 4, 4, 128], F32R,
                          kind="ExternalInput").ap()
    b1_d = nc.dram_tensor("b1", [128, 4], F32, kind="ExternalInput").ap()
    b2_d = nc.dram_tensor("b2", [128, 4], F32, kind="ExternalInput").ap()
    out_d = nc.dram_tensor("out", [2, LCH, N, N], F32,
                           kind="ExternalOutput").ap()

    ID = mybir.ActivationFunctionType.Identity

    with tile.TileContext(nc) as tc:
        with ExitStack() as ctx:
            wpool = ctx.enter_context(tc.tile_pool(name="w", bufs=1))
            bpool = ctx.enter_context(tc.tile_pool(name="bias", bufs=1))
            ptpool = ctx.enter_context(tc.tile_pool(name="pt", bufs=4))
            tpool = ctx.enter_context(tc.tile_pool(name="t", bufs=4))
            combpool = ctx.enter_context(tc.tile_pool(name="comb", bufs=3))
            ppsum = ctx.enter_context(
                tc.tile_pool(name="pp", bufs=4, space="PSUM"))
            lpsum = ctx.enter_context(
                tc.tile_pool(name="lp", bufs=2, space="PSUM"))
            opool = ctx.enter_context(tc.tile_pool(name="o", bufs=4))

            # Weights resident in SBUF: [p_in, (pc, cc, c_in)] - one 1MiB DMA
            w_t, b_t = [], []
            for s, (wd, bd) in enumerate(((w1_d, b1_d), (w2_d, b2_d))):
                w = wpool.tile([128, 16 * 128], F32R, tag=f"w{s}")
                nc.sync.dma_start(w[:], wd.rearrange("p a b c -> p (a b c)"))
                b = bpool.tile([128, 128], F32, tag=f"b{s}")
                nc.sync.dma_start(b[:, :4], bd)
                w_t.append(w)
                b_t.append(b)

            # t_tiles: ii -> (tile, base free offset); tiles cover 4 ii each
            t_tiles = {}

            def alloc_comb():
                # [c_in, (j, s, m)] per cc chunk; j = ii%4, s = step
                return [combpool.tile([128, 1024], lneg_dt, tag=f"comb{cc}",
                                        name=f"comb{cc}")
                        for cc in range(4)]

            def comb_view(ct):
                # [c_in, j, s, m]
                return ct[:].rearrange("c (j s m) -> c j s m", j=4, s=2)

            def emit_lneg(g, comb, n_groups):
                """All-pairs for ii-group g: stationary t[ii], moving 256
                (s1 pred l=ii | s2 pred l=ii-1). Results:
                s1 -> out[0, 4g..4g+3], s2 -> out[1, 4g-1..4g+2]."""
                lp = lpsum.tile([128, 1024], F32, tag="lp")
                for j in range(4):
                    tt, base = t_tiles[4 * g + j]
                    for cc in range(4):
                        nc.tensor.matmul(
                            lp[:, bass.ds(j * 256, 256)],
                            tt[:, bass.ds(base + cc * 128, 128)],
                            comb[cc][:, bass.ds(j * 256, 256)],
                            start=(cc == 0),
                            stop=(cc == 3),
                        )
                ot = opool.tile([128, 1024], F32, tag="o")
                nc.vector.tensor_scalar_mul(ot[:], lp[:], 1.0 / TEMP)
                otv = ot[:].rearrange("n (j s m) -> n j s m", j=4, s=2)
                nc.gpsimd.dma_start(
                    out_d[0, bass.ts(g, 4), :, :].rearrange("l n m -> n l m"),
                    otv[:, :, 0, :],
                )
                if g == 0:   # s2 l = -1..2 -> keep l = 0..2
                    nc.gpsimd.dma_start(
                        out_d[1, bass.ds(0, 3), :, :].rearrange(
                            "l n m -> n l m"),
                        otv[:, 1:4, 1, :],
                    )
                else:        # s2 l = 4g-1..4g+2
                    nc.gpsimd.dma_start(
                        out_d[1, bass.ds(4 * g - 1, 4), :, :].rearrange(
                            "l n m -> n l m"),
                        otv[:, :, 1, :],
                    )

            def emit_body():
                comb_cur = alloc_comb()       # group g=0
                # g=0 (j=0, s=1) slot is pred2[l=-1]: never written, its
                # output is discarded; zero it so matmuls see finite data.
                for cc in range(4):
                    z = comb_cur[cc][:, bass.ds(128, 128)]
                    if lneg_dt == F32R:
                        z = z.bitcast(F32)
                    nc.vector.memset(z, 0.0)
                prev = None                   # comb of group lb-1

                for lb in range(n_lb):
                    # one 1MiB load: patient block [p_in, (pc, l4, m)]
                    pt4 = ptpool.tile([128, 2048], F32R, tag="pt")
                    nc.sync.dma_start(
                        pt4[:], pt_d[lb].rearrange("p a l m -> p (a l m)"))
                    # one 1MiB load: query timesteps ii = 4lb..4lb+3,
                    # [c_in, (i, cc, n)]
                    tt4 = tpool.tile([128, 2048], lneg_dt, tag="t")
                    nc.scalar.dma_start(
                        tt4[:].rearrange("p (i a n) -> p i a n", i=4, a=4),
                        ts_d[bass.ts(lb, 4)].rearrange("i p a n -> p i a n"),
                    )
                    for i in range(4):
                        t_tiles[4 * lb + i] = (tt4, i * 512)

                    comb_next = alloc_comb()  # group lb+1

                    # pred for this block into comb tiles
                    for s in range(2):
                        for cc in range(4):
                            ps = ppsum.tile([128, 512], F32, tag="pp")
                            for pc in range(4):
                                nc.tensor.matmul(
                                    ps[:],
                                    w_t[s][:, bass.ts(pc * 4 + cc, 128)],
                                    pt4[:, bass.ts(pc, 512)],
                                    start=(pc == 0),
                                    stop=(pc == 3),
                                )
                            psv = ps[:].rearrange("c (j m) -> c j m", j=4)
                            bias = b_t[s][:, cc:cc + 1]
                            if s == 0:
                                # pred1[l=4lb+j] -> comb_cur (j, s=0)
                                # cc split across ACT/DVE to balance engines
                                if cc % 2 == 0:
                                    nc.scalar.activation(
                                        comb_view(comb_cur[cc])[:, :, 0, :],
                                        psv, ID, bias=bias)
                                else:
                                    nc.vector.tensor_scalar_add(
                                        comb_view(comb_cur[cc])[:, :, 0, :],
                                        psv, bias)
                            else:
                                # pred2[l=4lb+j'] -> j'=0..2: comb_cur
                                # (j'+1, s=1); j'=3: comb_next (0, s=1)
                                nc.scalar.activation(
                                    comb_view(comb_cur[cc])[:, 1:4, 1, :],
                                    psv[:, 0:3, :], ID, bias=bias)
                                nc.scalar.activation(
                                    comb_view(comb_next[cc])[:, 0:1, 1, :],
                                    psv[:, 3:4, :], ID, bias=bias)

                    # all-pairs for previous group (software pipeline skew)
                    if prev is not None:
                        emit_lneg(lb - 1, prev, n_lb)
                    prev = comb_cur
                    comb_cur = comb_next

                emit_lneg(n_lb - 1, prev, n_lb)

                # tail: (s2, l = 4*n_lb - 1) = t[ii=4*n_lb] x pred2 in
                # comb_cur (j=0, s=1)
                tt4h = tpool.tile([128, 2048], lneg_dt, tag="t")
                nc.scalar.dma_start(
                    tt4h[:].rearrange("p (i a n) -> p i a n", i=4, a=4),
                    ts_d[bass.ds(4 * n_lb, 4)].rearrange("i p a n -> p i a n"),
                )
                lp = lpsum.tile([128, 1024], F32, tag="lp")
                for cc in range(4):
                    nc.tensor.matmul(
                        lp[:, bass.ds(0, 128)],
                        tt4h[:, bass.ds(cc * 128, 128)],
                        comb_cur[cc][:, bass.ds(128, 128)],
                        start=(cc == 0),
                        stop=(cc == 3),
                    )
                ot = opool.tile([128, 1024], F32, tag="o")
                nc.vector.tensor_scalar_mul(
                    ot[:, :128], lp[:, :128], 1.0 / TEMP)
                nc.gpsimd.dma_start(out_d[1, 4 * n_lb - 1, :, :],
                                    ot[:, :128])

            if loop_n is None:
                emit_body()
            else:
                with tc.For_i(0, loop_n, 1):
                    emit_body()

    nc.compile()
    return nc


def get_program(lneg_dt_name=LNEG_DT, n_lb=LB, loop_n=None):
    key = (lneg_dt_name, n_lb, loop_n)
    if key not in _prog_cache:
        _prog_cache[key] = _build_program(lneg_dt_name, n_lb, loop_n)
    return _prog_cache[key]


def prep_inputs(timesteps, patient_timesteps, W1, b1, W2, b2,
                lneg_dt_name=LNEG_DT):
    """Shard + lay out the full inputs into per-core in_maps."""
    import ml_dtypes
    ts_np = (np.float32 if lneg_dt_name == "f32r" else ml_dtypes.bfloat16)
    timesteps = np.asarray(timesteps, np.float32)
    patient = np.asarray(patient_timesteps, np.float32)

    def w_prep(W):
        # W [c, p] -> [p_in, pc, cc, c_in]
        return np.ascontiguousarray(
            np.asarray(W, np.float32).T.reshape(4, 128, 4, 128)
            .transpose(1, 0, 2, 3))

    def b_prep(b):
        # b [c] -> [c_in, cc]
        return np.ascontiguousarray(
            np.asarray(b, np.float32).reshape(4, 128).T)

    w1p, w2p, b1p, b2p = w_prep(W1), w_prep(W2), b_prep(b1), b_prep(b2)

    in_maps = []
    for k in range(NCORES):
        lk = k * LCH
        # patient slice -> [lb, p_in, pc, l4, m]
        a = patient[:, lk:lk + LCH, :]            # [m, l, p]
        a = a.reshape(N, LB, 4, 4, 128)           # [m, lb, l4, pc, p_in]
        pt5 = np.ascontiguousarray(a.transpose(1, 4, 3, 2, 0))
        # timesteps halo slice -> [ii, c_in, cc, n]; ii = t_idx - lk - 1
        b = timesteps[:, lk + 1:lk + LCH + 2, :]  # [n, <=129, c]
        if b.shape[1] < LCH + 4:                  # pad to LCH+4 for 4-wide DMA
            pad = np.zeros((N, LCH + 4 - b.shape[1], C), np.float32)
            b = np.concatenate([b, pad], axis=1)
        b = b.reshape(N, LCH + 4, 4, 128)         # [n, ii, cc, c_in]
        ts4 = np.ascontiguousarray(b.transpose(1, 3, 2, 0)).astype(ts_np)
        in_maps.append({
            "pt": pt5, "ts": ts4,
            "w1": w1p, "w2": w2p, "b1": b1p, "b2": b2p,
        })
    return in_maps


def postprocess(core_outs):
    """core_outs: list of per-core "out" arrays [2, LCH, N, N] (already /T).

    Returns (logits [N, N, 2L-3], y_trues [N, 2L-3] int32, accs [2] f32).
    """
    full = np.concatenate([co for co in core_outs], axis=1)  # [2, L, N, N]
    s1 = full[0, :L - 1]   # [L-1, n, m]
    s2 = full[1, :L - 2]   # [L-2, n, m]

    accs = []
    idx = np.arange(N)
    for arr in (s1, s2):
        pos = arr[:, idx, idx]                     # [L', n]
        greater = arr < pos[:, :, None]            # pos > neg, [L', n, m]
        greater[:, idx, idx] = True
        ok = greater.all(axis=2)                   # [L', n]
        accs.append(ok.mean(dtype=np.float64))
    accs = np.array(accs, np.float32)

    logits = np.concatenate(
        [s1.transpose(1, 2, 0), s2.transpose(1, 2, 0)], axis=2)
    logits = np.ascontiguousarray(logits, dtype=np.float32)
    y_trues = np.broadcast_to(idx[:, None], (N, logits.shape[2]))
    y_trues = np.ascontiguousarray(y_trues, dtype=np.int32)
    return logits, y_trues, accs


def kernel(timesteps, patient_timesteps, W1, b1, W2, b2):
    nc = get_program()
    in_maps = prep_inputs(timesteps, patient_timesteps, W1, b1, W2, b2)
    res = bass_utils.run_bass_kernel_spmd(
        nc, in_maps, core_ids=list(range(NCORES)))
    core_outs = [res.results[k]["out"] for k in range(NCORES)]
    return postprocess(core_outs)
